# revision 8
# baseline (speedup 1.0000x reference)
"""Multi-head self-attention on 8 Trainium2 NeuronCores (Bass/Tile).

Problem: x[2,2048,1024] -> MHA(16 heads, d_head 64) -> out[2,2048,1024].

Sharding (batch x head-group, Megatron-ish, collective-free):
  core c (0..7): batch b = c//4, head group g = c%4 (heads 4g..4g+3).
  Each core computes q/k/v projections for its 4 heads over its batch,
  attention for those heads, and a PARTIAL output projection
  attn_local[256ch] @ w_out[256ch rows] over the full sequence. The host
  sums the 4 partials per batch (the Megatron row-parallel all-reduce is
  folded into the unshard step; b_out and the V-bias term bv @ w_out are
  added once on the host -- exact, since softmax rows sum to 1).

On-core layout (TensorE compute in bf16, fp32 PSUM accumulation):
  - ACT (exp for softmax) is the bottleneck engine: 16.8M exps/core ~=
    147us of ACT time. The schedule is built to saturate ACT from ~5us:
    input DMA is split across four engine HWDGE queues with x^T loaded
    token-sliced, and k/q chunk-0 projections are emitted first so
    scores round 0 feeds ACT immediately; V and the remaining q/k
    projections run on the PE behind ACT pacing.
  - q/k/v projections use K=64 row-split matmul pairs (partitions 0-63 /
    64-127 accumulate into the same PSUM tile) so the two streams run in
    disjoint PE row groups and each stream's LDWEIGHTS hides behind the
    other's matmul.
  - qT/kT in [channel, t] layout: scores^T = kT.T @ qT needs no
    transposes; the two heads of a 128-channel chunk sit in partitions
    0-63/64-127 so their K=64 score matmuls run concurrently.
  - softmax: scores^T [128ki, qi] tiles -> ACT exp (PSUM->SBUF bf16,
    scale=1/8 folded, no max subtraction: |s|/8 <= ~2). p-tiles are
    4-deep buffered so round r+1's exp never waits on round r's PV.
  - PV: attn^T = V.T @ P~ as column-tiled concurrent M=64 head pairs;
    softmax denominators via a DVE bf16 add-tree plus a K=128
    ones-matmul; reciprocal_approx_fast + K=1 ones-matmul broadcast,
    pipelined one round behind PV; out-projection chunks per query group.
"""

import os
import numpy as np
import ml_dtypes

# bisect toggles (read at build time)
DMA_MULTI = os.environ.get("K_DMA_MULTI", "1") == "1"
KSPLIT = os.environ.get("K_KSPLIT", "1") == "1"

import concourse.bass as bass
import concourse.mybir as mybir
import concourse.tile as tile
from concourse import bacc
from concourse import bass_utils
from concourse.bass import ts

BF = mybir.dt.bfloat16
F32 = mybir.dt.float32

B, T, C = 2, 2048, 1024
H, DH = 16, 64
N_CORES = 8
HG = 4  # heads per core
CH = HG * DH  # 256 channels per core

LAST_RESULT = None  # BassKernelResults of the most recent run (for profiling)
_NC_CACHE = None


def _build_nc():
    nc = bacc.Bacc(
        "TRN2", target_bir_lowering=False, debug=False, num_devices=N_CORES
    )

    xt = nc.dram_tensor("xt", [C, T], BF, kind="ExternalInput")
    wq = nc.dram_tensor("wq", [C, CH], BF, kind="ExternalInput")
    wk = nc.dram_tensor("wk", [C, CH], BF, kind="ExternalInput")
    wv = nc.dram_tensor("wv", [C, CH], BF, kind="ExternalInput")
    bqt = nc.dram_tensor("bqt", [128, 2], F32, kind="ExternalInput")
    bkt = nc.dram_tensor("bkt", [128, 2], F32, kind="ExternalInput")
    wout = nc.dram_tensor("wout", [CH, C], BF, kind="ExternalInput")
    out = nc.dram_tensor("out", [T, C], F32, kind="ExternalOutput")

    with tile.TileContext(nc) as tc:
        with (
            tc.tile_pool(name="persist", bufs=1) as persist,
            tc.tile_pool(name="consts", bufs=1) as consts,
            tc.tile_pool(name="sbn", bufs=6) as sbn,
            tc.tile_pool(name="osb", bufs=3) as osb,
            tc.tile_pool(name="ps_st", bufs=2, space="PSUM") as ps_st,
            tc.tile_pool(name="ps_pv", bufs=2, space="PSUM") as ps_pv,
            tc.tile_pool(name="ps_misc", bufs=2, space="PSUM") as ps_misc,
        ):
            ones_bf = consts.tile([1, 128], BF)
            nc.vector.memset(ones_bf[:], 1.0)
            ones_col = consts.tile([128, 1], BF)
            nc.vector.memset(ones_col[:], 1.0)

            xT = persist.tile([128, 8, T], BF, tag="xT")
            wq_sb = persist.tile([128, 8, CH], BF, tag="wq")
            wk_sb = persist.tile([128, 8, CH], BF, tag="wk")
            wv_sb = persist.tile([128, 8, CH], BF, tag="wv")
            wout_sb = persist.tile([128, 2, C], BF, tag="wout")
            bqt_sb = consts.tile([128, 2], F32)
            bkt_sb = consts.tile([128, 2], F32)

            # ---- input DMA on four parallel HWDGE queues ----
            # sync: wk (first compute dep); vector: wq; gpsimd: x^T
            # token-sliced; scalar: biases + wv + wout (later deps).
            xt_r = xt.rearrange("(ci p) t -> p ci t", p=128)
            if DMA_MULTI:
                nc.sync.dma_start(
                    out=wk_sb[:], in_=wk.rearrange("(ci p) j -> p ci j", p=128)
                )
                for tt in range(4):
                    nc.gpsimd.dma_start(
                        out=xT[:, :, ts(tt, 512)], in_=xt_r[:, :, ts(tt, 512)]
                    )
                nc.scalar.dma_start(out=bkt_sb[:], in_=bkt[:])
                nc.scalar.dma_start(out=bqt_sb[:], in_=bqt[:])
                nc.scalar.dma_start(
                    out=wq_sb[:], in_=wq.rearrange("(ci p) j -> p ci j", p=128)
                )
                nc.sync.dma_start(
                    out=wv_sb[:], in_=wv.rearrange("(ci p) j -> p ci j", p=128)
                )
                nc.sync.dma_start(
                    out=wout_sb[:], in_=wout.rearrange("(hp p) j -> p hp j", p=128)
                )
            else:
                nc.sync.dma_start(
                    out=wk_sb[:], in_=wk.rearrange("(ci p) j -> p ci j", p=128)
                )
                nc.sync.dma_start(out=bkt_sb[:], in_=bkt[:])
                nc.sync.dma_start(out=bqt_sb[:], in_=bqt[:])
                nc.sync.dma_start(
                    out=wq_sb[:], in_=wq.rearrange("(ci p) j -> p ci j", p=128)
                )
                for tt in range(4):
                    nc.sync.dma_start(
                        out=xT[:, :, ts(tt, 512)], in_=xt_r[:, :, ts(tt, 512)]
                    )
                nc.sync.dma_start(
                    out=wv_sb[:], in_=wv.rearrange("(ci p) j -> p ci j", p=128)
                )
                nc.sync.dma_start(
                    out=wout_sb[:], in_=wout.rearrange("(hp p) j -> p hp j", p=128)
                )

            # ---- persistent activations ----
            # qkT[:, 0:2, :] = qT chunks (hp), [:, 2:4, :] = kT chunks;
            # chunk hp rows 0-63 = head 2hp, rows 64-127 = head 2hp+1.
            qkT = persist.tile([128, 4, T], BF, tag="qkT")
            vext = persist.tile([128, T // 128, HG, DH], BF, tag="vext")
            attn_p = [
                [
                    persist.tile(
                        [128, 512], BF, tag=f"attnp{hp}_{qg}",
                        name=f"attnp{hp}_{qg}",
                    )
                    for qg in range(4)
                ]
                for hp in range(2)
            ]

            def qk_group(w_i, co, tt):
                """one [128,512] tile of qT (w_i=0) or kT (w_i=1), chunk co.

                K=64 row-split pairs: the 0-63 / 64-127 partition halves
                accumulate into the same PSUM tile from disjoint PE row
                groups, hiding each stream's LDWEIGHTS behind the other.
                """
                wsb = wq_sb if w_i == 0 else wk_sb
                bias_sb = bqt_sb if w_i == 0 else bkt_sb
                qp = ps_misc.tile([128, 512], F32, tag="sm", name="qp")
                if KSPLIT:
                    for ci in range(8):
                        nc.tensor.matmul(
                            qp[:],
                            wsb[0:64, ci, ts(co, 128)],
                            xT[0:64, ci, ts(tt, 512)],
                            start=(ci == 0),
                            stop=False,
                        )
                        nc.tensor.matmul(
                            qp[:],
                            wsb[64:128, ci, ts(co, 128)],
                            xT[64:128, ci, ts(tt, 512)],
                            start=False,
                            stop=(ci == 7),
                        )
                else:
                    for ci in range(8):
                        nc.tensor.matmul(
                            qp[:],
                            wsb[:, ci, ts(co, 128)],
                            xT[:, ci, ts(tt, 512)],
                            start=(ci == 0),
                            stop=(ci == 7),
                        )
                nc.scalar.add(
                    qkT[:, 2 * w_i + co, ts(tt, 512)],
                    qp[:],
                    bias_sb[:, co : co + 1],
                )

            def v_group(tt):
                vp = ps_misc.tile([128, CH], F32, tag="sm", name="vp")
                if KSPLIT:
                    for ci in range(8):
                        nc.tensor.matmul(
                            vp[:],
                            xT[0:64, ci, ts(tt, 128)],
                            wv_sb[0:64, ci, :],
                            start=(ci == 0),
                            stop=False,
                        )
                        nc.tensor.matmul(
                            vp[:],
                            xT[64:128, ci, ts(tt, 128)],
                            wv_sb[64:128, ci, :],
                            start=False,
                            stop=(ci == 7),
                        )
                else:
                    for ci in range(8):
                        nc.tensor.matmul(
                            vp[:],
                            xT[:, ci, ts(tt, 128)],
                            wv_sb[:, ci, :],
                            start=(ci == 0),
                            stop=(ci == 7),
                        )
                nc.vector.tensor_copy(
                    vext[:, tt, :, :],
                    vp[:].rearrange("p (h d) -> p h d", h=HG),
                )

            p_tiles = {}
            rec_tiles = {}
            tmp_tiles = {}

            def p_alloc(qg, hp):
                pa = osb.tile([128, 8, 1024], BF, tag="p", bufs=4, name="pa")
                pb = osb.tile([128, 8, 1024], BF, tag="p", bufs=4, name="pb")
                p_tiles[(qg, hp)] = (pa, pb)
                return pa, pb

            def st_seg(qg, hp, kps, pa, pb):
                """scores^T + exp for head pair hp, query group qg, kp range."""
                qs = ts(qg, 512)
                for kp in kps:
                    stA = ps_st.tile([128, 1024], F32, tag="st", name="stA")
                    stB = ps_st.tile([128, 1024], F32, tag="st", name="stB")
                    for j in range(2):
                        ki = 2 * kp + j
                        nc.tensor.matmul(
                            stA[:, ts(j, 512)],
                            qkT[0:64, 2 + hp, ts(ki, 128)],
                            qkT[0:64, hp, qs],
                            start=True, stop=True,
                        )
                        nc.tensor.matmul(
                            stB[:, ts(j, 512)],
                            qkT[64:128, 2 + hp, ts(ki, 128)],
                            qkT[64:128, hp, qs],
                            start=True, stop=True,
                        )
                    nc.scalar.activation(
                        pa[:, kp, :], stA[:],
                        mybir.ActivationFunctionType.Exp, scale=1.0 / 8.0,
                    )
                    nc.scalar.activation(
                        pb[:, kp, :], stB[:],
                        mybir.ActivationFunctionType.Exp, scale=1.0 / 8.0,
                    )

            def st_part(qg, hp):
                pa, pb = p_alloc(qg, hp)
                st_seg(qg, hp, range(8), pa, pb)

            def pv_part(qg, hp):
                pa, pb = p_tiles.pop((qg, hp))
                # denominator add-trees first: they depend only on the exps,
                # so emitting them ahead of the PV matmuls keeps the DVE
                # queue from head-blocking on the PV-dependent tmp copy.
                t4s = {}
                for hh, pbuf in ((0, pa), (1, pb)):
                    t1 = sbn.tile([128, 4, 1024], BF, tag="t1", name="t1", bufs=2)
                    nc.vector.tensor_add(
                        t1[:], pbuf[:, 0:4, :], pbuf[:, 4:8, :]
                    )
                    t2 = sbn.tile([128, 2, 1024], BF, tag="t2", name="t2", bufs=2)
                    nc.vector.tensor_add(
                        t2[:], t1[:, 0:2, :], t1[:, 2:4, :]
                    )
                    t3 = sbn.tile([128, 1024], BF, tag="t3", name="t3", bufs=2)
                    nc.vector.tensor_add(
                        t3[:], t2[:, 0, :], t2[:, 1, :]
                    )
                    t4 = sbn.tile([128, 512], BF, tag="t4", name="t4", bufs=2)
                    nc.vector.tensor_add(
                        t4[:], t3[:, 0:512], t3[:, 512:1024]
                    )
                    t4s[hh] = t4
                # paired PV: head 2hp -> psum partitions 0-63 (col group 0-1),
                # head 2hp+1 -> partitions 64-127 (col group 2-3); the two
                # column-tiled matmul streams run concurrently on the PE.
                pv = ps_pv.tile([128, 512], F32, tag="pv", name="pv")
                for ki in range(16):
                    for hh, pbuf in ((0, pa), (1, pb)):
                        h = 2 * hp + hh
                        nc.tensor.matmul(
                            pv[64 * hh : 64 * hh + 64, :],
                            vext[:, ki, h, :],
                            pbuf[:, ki // 2, ts(ki % 2, 512)],
                            start=(ki == 0),
                            stop=(ki == 15),
                        )
                # partition-axis fold of the partial denominators (K=128
                # ones-matmul), then the reciprocal chain (DVE-only)
                for hh in range(2):
                    h = 2 * hp + hh
                    dps = ps_misc.tile([128, 512], F32, tag="sm", name="dps")
                    nc.tensor.matmul(
                        dps[0:1, :], ones_col[:, 0:1], t4s[hh][:],
                        start=True, stop=True,
                    )
                    rec32 = sbn.tile([1, 512], F32, tag="rec32", name="rc", bufs=4)
                    nc.vector.tensor_copy(rec32[:], dps[0:1, :])
                    nc.vector.reciprocal_approx_fast(out=rec32[:], in_=rec32[:])
                    rec_bf = sbn.tile([1, 512], BF, tag="rec", name="rb")
                    nc.vector.tensor_copy(rec_bf[:], rec32[:])
                    rec_tiles[4 * qg + h] = rec_bf
                tmp = sbn.tile([128, 512], F32, tag="tmp", name="tmp", bufs=4)
                nc.vector.tensor_copy(tmp[:], pv[:])
                tmp_tiles[(qg, hp)] = tmp

            def normalize_round(qg, hp):
                """rep-matmul + multiply -> attn_p[hp][qg] (both heads)."""
                rp = ps_misc.tile([128, 512], F32, tag="sm", name="rp")
                tmp = tmp_tiles.pop((qg, hp))
                for hh in range(2):
                    slot = 4 * qg + 2 * hp + hh
                    rows = slice(64 * hh, 64 * hh + 64)
                    nc.tensor.matmul(
                        rp[rows, :], ones_bf[0:1, 0:64], rec_tiles[slot][:],
                        start=True, stop=True,
                    )
                    nc.vector.tensor_mul(
                        attn_p[hp][qg][rows, :],
                        tmp[rows, :],
                        rp[rows, :],
                    )

            def outproj_chunk(qg):
                """partial out-projection rows for query group qg."""
                for tt4 in range(4):
                    tt = 4 * qg + tt4
                    o_sb = osb.tile([128, C], F32, tag="o", name="osb", bufs=2)
                    for cn in range(2):
                        op = ps_misc.tile(
                            [128, 512], F32, tag="sm", name="op"
                        )
                        for hp in range(2):
                            nc.tensor.matmul(
                                op[:],
                                attn_p[hp][qg][:, ts(tt4, 128)],
                                wout_sb[:, hp, ts(cn, 512)],
                                start=(hp == 0),
                                stop=(hp == 1),
                            )
                        nc.vector.tensor_copy(o_sb[:, ts(cn, 512)], op[:])
                    nc.sync.dma_start(out=out[ts(tt, 128), :], in_=o_sb[:])

            # ---- flash-style startup: feed ACT as early as possible ----
            rounds = [(qg, hp) for qg in range(4) for hp in range(2)]

            # round 0 (qg0, hp0): k chunk0 + q chunk0(tt0), scores chase the
            # k tt-groups as they land.
            qk_group(1, 0, 0)
            qk_group(0, 0, 0)
            pa0, pb0 = p_alloc(0, 0)
            st_seg(0, 0, [0, 1], pa0, pb0)
            qk_group(1, 0, 1)
            st_seg(0, 0, [2, 3], pa0, pb0)
            qk_group(1, 0, 2)
            st_seg(0, 0, [4, 5], pa0, pb0)
            qk_group(1, 0, 3)
            st_seg(0, 0, [6, 7], pa0, pb0)

            # round 1 (qg0, hp1): k chunk1 + q chunk1(tt0)
            qk_group(1, 1, 0)
            qk_group(0, 1, 0)
            pa1, pb1 = p_alloc(0, 1)
            st_seg(0, 1, [0, 1], pa1, pb1)
            qk_group(1, 1, 1)
            st_seg(0, 1, [2, 3], pa1, pb1)
            qk_group(1, 1, 2)
            st_seg(0, 1, [4, 5], pa1, pb1)
            qk_group(1, 1, 3)
            st_seg(0, 1, [6, 7], pa1, pb1)

            # rounds 2,3 (qg1): q chunks tt1, then scores; keeps ACT fed
            # while V projections run on the PE behind it.
            qk_group(0, 0, 1)
            qk_group(0, 1, 1)
            st_part(1, 0)
            st_part(1, 1)
            for tt in range(16):
                v_group(tt)

            # ---- pipelined main stream ----
            for r, (qg, hp) in enumerate(rounds):
                pv_part(qg, hp)
                if r + 2 < len(rounds):
                    nqg, nhp = rounds[r + 2]
                    if nhp == 0 and nqg >= 2:
                        qk_group(0, 0, nqg)
                        qk_group(0, 1, nqg)
                    if nqg >= 2:
                        st_part(nqg, nhp)
                if r >= 1:
                    pqg, php = rounds[r - 1]
                    normalize_round(pqg, php)
                    if php == 1:
                        outproj_chunk(pqg)
            normalize_round(*rounds[-1])
            outproj_chunk(rounds[-1][0])

    nc.compile()
    return nc


def _get_nc():
    global _NC_CACHE
    if _NC_CACHE is None:
        _NC_CACHE = _build_nc()
    return _NC_CACHE


def kernel(x, w_qkv, b_qkv, w_out, b_out):
    global LAST_RESULT
    x = np.asarray(x, dtype=np.float32)
    w_qkv = np.asarray(w_qkv, dtype=np.float32)
    b_qkv = np.asarray(b_qkv, dtype=np.float32)
    w_out = np.asarray(w_out, dtype=np.float32)
    b_out = np.asarray(b_out, dtype=np.float32)

    bf = ml_dtypes.bfloat16
    in_maps = []
    for c in range(N_CORES):
        b, g = divmod(c, 4)
        cols = slice(CH * g, CH * (g + 1))
        bq = b_qkv[0 * C + CH * g : 0 * C + CH * (g + 1)]
        bk = b_qkv[1 * C + CH * g : 1 * C + CH * (g + 1)]
        in_maps.append(
            {
                "xt": np.ascontiguousarray(x[b].astype(bf).T),
                "wq": np.ascontiguousarray(w_qkv[:, 0 * C :][:, cols]).astype(bf),
                "wk": np.ascontiguousarray(w_qkv[:, 1 * C :][:, cols]).astype(bf),
                "wv": np.ascontiguousarray(w_qkv[:, 2 * C :][:, cols]).astype(bf),
                "bqt": np.ascontiguousarray(bq.reshape(2, 128).T),
                "bkt": np.ascontiguousarray(bk.reshape(2, 128).T),
                "wout": np.ascontiguousarray(w_out[CH * g : CH * (g + 1), :]).astype(bf),
            }
        )

    nc = _get_nc()
    LAST_RESULT = bass_utils.run_bass_kernel_spmd(
        nc, in_maps, core_ids=list(range(N_CORES))
    )

    full = np.zeros((B, T, C), dtype=np.float32)
    # bias folded once on the host: b_out plus the V-bias pushed through
    # w_out (normalized attention rows sum to 1, so bv contributes exactly
    # bv @ w_out to every token)
    full += b_out + b_qkv[2 * C : 3 * C] @ w_out
    for c in range(N_CORES):
        b = c // 4
        full[b] += LAST_RESULT.results[c]["out"]
    return full


# revision 11
# speedup vs baseline: 1.0100x; 1.0100x over previous
"""Multi-head self-attention on 8 Trainium2 NeuronCores (Bass/Tile).

Problem: x[2,2048,1024] -> MHA(16 heads, d_head 64) -> out[2,2048,1024].

Sharding (batch x head-group, Megatron-ish, collective-free):
  core c (0..7): batch b = c//4, head group g = c%4 (heads 4g..4g+3).
  Each core computes q/k/v projections for its 4 heads over its batch,
  attention for those heads, and a PARTIAL output projection
  attn_local[256ch] @ w_out[256ch rows] over the full sequence. The host
  sums the 4 partials per batch (the Megatron row-parallel all-reduce is
  folded into the unshard step; b_out and the V-bias term bv @ w_out are
  added once on the host -- exact, since softmax rows sum to 1).

On-core layout (TensorE compute in bf16, fp32 PSUM accumulation):
  - ACT (exp for softmax) is the bottleneck engine: 16.8M exps/core ~=
    147us of ACT time. The schedule is built to saturate ACT from ~5us:
    input DMA is split across four engine HWDGE queues with x^T loaded
    token-sliced, and k/q chunk-0 projections are emitted first so
    scores round 0 feeds ACT immediately; V and the remaining q/k
    projections run on the PE behind ACT pacing.
  - q/k/v projections use K=64 row-split matmul pairs (partitions 0-63 /
    64-127 accumulate into the same PSUM tile) so the two streams run in
    disjoint PE row groups and each stream's LDWEIGHTS hides behind the
    other's matmul.
  - qT/kT in [channel, t] layout: scores^T = kT.T @ qT needs no
    transposes; the two heads of a 128-channel chunk sit in partitions
    0-63/64-127 so their K=64 score matmuls run concurrently.
  - softmax: scores^T [128ki, qi] tiles -> ACT exp (PSUM->SBUF bf16,
    scale=1/8 folded, no max subtraction: |s|/8 <= ~2). p-tiles are
    4-deep buffered so round r+1's exp never waits on round r's PV.
  - PV: attn^T = V.T @ P~ as column-tiled concurrent M=64 head pairs;
    softmax denominators via a DVE bf16 add-tree plus a K=128
    ones-matmul; reciprocal_approx_fast + K=1 ones-matmul broadcast,
    pipelined one round behind PV; out-projection chunks per query group.
"""

import os
import numpy as np
import ml_dtypes

# bisect toggles (read at build time)
DMA_MULTI = os.environ.get("K_DMA_MULTI", "1") == "1"
KSPLIT = os.environ.get("K_KSPLIT", "0") == "1"

import concourse.bass as bass
import concourse.mybir as mybir
import concourse.tile as tile
from concourse import bacc
from concourse import bass_utils
from concourse.bass import ts

BF = mybir.dt.bfloat16
F32 = mybir.dt.float32

B, T, C = 2, 2048, 1024
H, DH = 16, 64
N_CORES = 8
HG = 4  # heads per core
CH = HG * DH  # 256 channels per core

LAST_RESULT = None  # BassKernelResults of the most recent run (for profiling)
_NC_CACHE = None


def _build_nc():
    nc = bacc.Bacc(
        "TRN2", target_bir_lowering=False, debug=False, num_devices=N_CORES
    )

    xt = nc.dram_tensor("xt", [C, T], BF, kind="ExternalInput")
    wq = nc.dram_tensor("wq", [C, CH], BF, kind="ExternalInput")
    wk = nc.dram_tensor("wk", [C, CH], BF, kind="ExternalInput")
    wv = nc.dram_tensor("wv", [C, CH], BF, kind="ExternalInput")
    bqt = nc.dram_tensor("bqt", [128, 2], F32, kind="ExternalInput")
    bkt = nc.dram_tensor("bkt", [128, 2], F32, kind="ExternalInput")
    wout = nc.dram_tensor("wout", [CH, C], BF, kind="ExternalInput")
    out = nc.dram_tensor("out", [T, C], F32, kind="ExternalOutput")

    with tile.TileContext(nc) as tc:
        with (
            tc.tile_pool(name="persist", bufs=1) as persist,
            tc.tile_pool(name="consts", bufs=1) as consts,
            tc.tile_pool(name="sbn", bufs=6) as sbn,
            tc.tile_pool(name="osb", bufs=3) as osb,
            tc.tile_pool(name="ps_st", bufs=2, space="PSUM") as ps_st,
            tc.tile_pool(name="ps_pv", bufs=2, space="PSUM") as ps_pv,
            tc.tile_pool(name="ps_misc", bufs=2, space="PSUM") as ps_misc,
        ):
            ones_bf = consts.tile([1, 128], BF)
            nc.vector.memset(ones_bf[:], 1.0)
            ones_col = consts.tile([128, 1], BF)
            nc.vector.memset(ones_col[:], 1.0)

            xT = persist.tile([128, 8, T], BF, tag="xT")
            wq_sb = persist.tile([128, 8, CH], BF, tag="wq")
            wk_sb = persist.tile([128, 8, CH], BF, tag="wk")
            wv_sb = persist.tile([128, 8, CH], BF, tag="wv")
            wout_sb = persist.tile([128, 2, C], BF, tag="wout")
            bqt_sb = consts.tile([128, 2], F32)
            bkt_sb = consts.tile([128, 2], F32)

            # ---- input DMA on four parallel HWDGE queues ----
            # sync: wk (first compute dep); vector: wq; gpsimd: x^T
            # token-sliced; scalar: biases + wv + wout (later deps).
            xt_r = xt.rearrange("(ci p) t -> p ci t", p=128)
            if DMA_MULTI:
                # two HWDGE queues (sync + scalar); x^T token-sliced so the
                # first k/q projections start after ~1.5MB instead of 5MB.
                nc.sync.dma_start(
                    out=wk_sb[:], in_=wk.rearrange("(ci p) j -> p ci j", p=128)
                )
                nc.scalar.dma_start(out=bkt_sb[:], in_=bkt[:])
                nc.scalar.dma_start(out=bqt_sb[:], in_=bqt[:])
                nc.scalar.dma_start(
                    out=wq_sb[:], in_=wq.rearrange("(ci p) j -> p ci j", p=128)
                )
                nc.sync.dma_start(out=xT[:, :, ts(0, 512)], in_=xt_r[:, :, ts(0, 512)])
                nc.scalar.dma_start(out=xT[:, :, ts(1, 512)], in_=xt_r[:, :, ts(1, 512)])
                nc.sync.dma_start(out=xT[:, :, ts(2, 512)], in_=xt_r[:, :, ts(2, 512)])
                nc.scalar.dma_start(out=xT[:, :, ts(3, 512)], in_=xt_r[:, :, ts(3, 512)])
                nc.scalar.dma_start(
                    out=wv_sb[:], in_=wv.rearrange("(ci p) j -> p ci j", p=128)
                )
                nc.sync.dma_start(
                    out=wout_sb[:], in_=wout.rearrange("(hp p) j -> p hp j", p=128)
                )
            else:
                nc.sync.dma_start(
                    out=wk_sb[:], in_=wk.rearrange("(ci p) j -> p ci j", p=128)
                )
                nc.sync.dma_start(out=bkt_sb[:], in_=bkt[:])
                nc.sync.dma_start(out=bqt_sb[:], in_=bqt[:])
                nc.sync.dma_start(
                    out=wq_sb[:], in_=wq.rearrange("(ci p) j -> p ci j", p=128)
                )
                for tt in range(4):
                    nc.sync.dma_start(
                        out=xT[:, :, ts(tt, 512)], in_=xt_r[:, :, ts(tt, 512)]
                    )
                nc.sync.dma_start(
                    out=wv_sb[:], in_=wv.rearrange("(ci p) j -> p ci j", p=128)
                )
                nc.sync.dma_start(
                    out=wout_sb[:], in_=wout.rearrange("(hp p) j -> p hp j", p=128)
                )

            # ---- persistent activations ----
            # qkT[:, 0:2, :] = qT chunks (hp), [:, 2:4, :] = kT chunks;
            # chunk hp rows 0-63 = head 2hp, rows 64-127 = head 2hp+1.
            qkT = persist.tile([128, 4, T], BF, tag="qkT")
            vext = persist.tile([128, T // 128, HG, DH], BF, tag="vext")
            attn_p = [
                [
                    persist.tile(
                        [128, 512], BF, tag=f"attnp{hp}_{qg}",
                        name=f"attnp{hp}_{qg}",
                    )
                    for qg in range(4)
                ]
                for hp in range(2)
            ]

            def qk_group(w_i, co, tt):
                """one [128,512] tile of qT (w_i=0) or kT (w_i=1), chunk co.

                K=64 row-split pairs: the 0-63 / 64-127 partition halves
                accumulate into the same PSUM tile from disjoint PE row
                groups, hiding each stream's LDWEIGHTS behind the other.
                """
                wsb = wq_sb if w_i == 0 else wk_sb
                bias_sb = bqt_sb if w_i == 0 else bkt_sb
                qp = ps_misc.tile([128, 512], F32, tag="sm", name="qp")
                if KSPLIT:
                    for ci in range(8):
                        nc.tensor.matmul(
                            qp[:],
                            wsb[0:64, ci, ts(co, 128)],
                            xT[0:64, ci, ts(tt, 512)],
                            start=(ci == 0),
                            stop=False,
                        )
                        nc.tensor.matmul(
                            qp[:],
                            wsb[64:128, ci, ts(co, 128)],
                            xT[64:128, ci, ts(tt, 512)],
                            start=False,
                            stop=(ci == 7),
                        )
                else:
                    for ci in range(8):
                        nc.tensor.matmul(
                            qp[:],
                            wsb[:, ci, ts(co, 128)],
                            xT[:, ci, ts(tt, 512)],
                            start=(ci == 0),
                            stop=(ci == 7),
                        )
                nc.scalar.add(
                    qkT[:, 2 * w_i + co, ts(tt, 512)],
                    qp[:],
                    bias_sb[:, co : co + 1],
                )

            def v_group(tt):
                vp = ps_misc.tile([128, CH], F32, tag="sm", name="vp")
                if KSPLIT:
                    for ci in range(8):
                        nc.tensor.matmul(
                            vp[:],
                            xT[0:64, ci, ts(tt, 128)],
                            wv_sb[0:64, ci, :],
                            start=(ci == 0),
                            stop=False,
                        )
                        nc.tensor.matmul(
                            vp[:],
                            xT[64:128, ci, ts(tt, 128)],
                            wv_sb[64:128, ci, :],
                            start=False,
                            stop=(ci == 7),
                        )
                else:
                    for ci in range(8):
                        nc.tensor.matmul(
                            vp[:],
                            xT[:, ci, ts(tt, 128)],
                            wv_sb[:, ci, :],
                            start=(ci == 0),
                            stop=(ci == 7),
                        )
                nc.vector.tensor_copy(
                    vext[:, tt, :, :],
                    vp[:].rearrange("p (h d) -> p h d", h=HG),
                )

            p_tiles = {}
            rec_tiles = {}
            tmp_tiles = {}

            def p_alloc(qg, hp):
                pa = osb.tile([128, 8, 1024], BF, tag="p", bufs=4, name="pa")
                pb = osb.tile([128, 8, 1024], BF, tag="p", bufs=4, name="pb")
                p_tiles[(qg, hp)] = (pa, pb)
                return pa, pb

            def st_seg(qg, hp, kps, pa, pb):
                """scores^T + exp for head pair hp, query group qg, kp range."""
                qs = ts(qg, 512)
                for kp in kps:
                    stA = ps_st.tile([128, 1024], F32, tag="st", name="stA")
                    stB = ps_st.tile([128, 1024], F32, tag="st", name="stB")
                    for j in range(2):
                        ki = 2 * kp + j
                        nc.tensor.matmul(
                            stA[:, ts(j, 512)],
                            qkT[0:64, 2 + hp, ts(ki, 128)],
                            qkT[0:64, hp, qs],
                            start=True, stop=True,
                        )
                        nc.tensor.matmul(
                            stB[:, ts(j, 512)],
                            qkT[64:128, 2 + hp, ts(ki, 128)],
                            qkT[64:128, hp, qs],
                            start=True, stop=True,
                        )
                    nc.scalar.activation(
                        pa[:, kp, :], stA[:],
                        mybir.ActivationFunctionType.Exp, scale=1.0 / 8.0,
                    )
                    nc.scalar.activation(
                        pb[:, kp, :], stB[:],
                        mybir.ActivationFunctionType.Exp, scale=1.0 / 8.0,
                    )

            def st_part(qg, hp):
                pa, pb = p_alloc(qg, hp)
                st_seg(qg, hp, range(8), pa, pb)

            def pv_part(qg, hp):
                pa, pb = p_tiles.pop((qg, hp))
                # denominator add-trees first: they depend only on the exps,
                # so emitting them ahead of the PV matmuls keeps the DVE
                # queue from head-blocking on the PV-dependent tmp copy.
                t4s = {}
                for hh, pbuf in ((0, pa), (1, pb)):
                    t1 = sbn.tile([128, 4, 1024], BF, tag="t1", name="t1", bufs=2)
                    nc.vector.tensor_add(
                        t1[:], pbuf[:, 0:4, :], pbuf[:, 4:8, :]
                    )
                    t2 = sbn.tile([128, 2, 1024], BF, tag="t2", name="t2", bufs=2)
                    nc.vector.tensor_add(
                        t2[:], t1[:, 0:2, :], t1[:, 2:4, :]
                    )
                    t3 = sbn.tile([128, 1024], BF, tag="t3", name="t3", bufs=2)
                    nc.vector.tensor_add(
                        t3[:], t2[:, 0, :], t2[:, 1, :]
                    )
                    t4 = sbn.tile([128, 512], BF, tag="t4", name="t4", bufs=2)
                    nc.vector.tensor_add(
                        t4[:], t3[:, 0:512], t3[:, 512:1024]
                    )
                    t4s[hh] = t4
                # paired PV: head 2hp -> psum partitions 0-63 (col group 0-1),
                # head 2hp+1 -> partitions 64-127 (col group 2-3); the two
                # column-tiled matmul streams run concurrently on the PE.
                pv = ps_pv.tile([128, 512], F32, tag="pv", name="pv")
                for ki in range(16):
                    for hh, pbuf in ((0, pa), (1, pb)):
                        h = 2 * hp + hh
                        nc.tensor.matmul(
                            pv[64 * hh : 64 * hh + 64, :],
                            vext[:, ki, h, :],
                            pbuf[:, ki // 2, ts(ki % 2, 512)],
                            start=(ki == 0),
                            stop=(ki == 15),
                        )
                # partition-axis fold of the partial denominators (K=128
                # ones-matmul), then the reciprocal chain (DVE-only)
                for hh in range(2):
                    h = 2 * hp + hh
                    dps = ps_misc.tile([128, 512], F32, tag="sm", name="dps")
                    nc.tensor.matmul(
                        dps[0:1, :], ones_col[:, 0:1], t4s[hh][:],
                        start=True, stop=True,
                    )
                    rec32 = sbn.tile([1, 512], F32, tag="rec32", name="rc", bufs=4)
                    nc.vector.tensor_copy(rec32[:], dps[0:1, :])
                    nc.vector.reciprocal_approx_fast(out=rec32[:], in_=rec32[:])
                    rec_bf = sbn.tile([1, 512], BF, tag="rec", name="rb")
                    nc.vector.tensor_copy(rec_bf[:], rec32[:])
                    rec_tiles[4 * qg + h] = rec_bf
                tmp = sbn.tile([128, 512], F32, tag="tmp", name="tmp", bufs=4)
                nc.vector.tensor_copy(tmp[:], pv[:])
                tmp_tiles[(qg, hp)] = tmp

            def normalize_round(qg, hp):
                """rep-matmul + multiply -> attn_p[hp][qg] (both heads)."""
                rp = ps_misc.tile([128, 512], F32, tag="sm", name="rp")
                tmp = tmp_tiles.pop((qg, hp))
                for hh in range(2):
                    slot = 4 * qg + 2 * hp + hh
                    rows = slice(64 * hh, 64 * hh + 64)
                    nc.tensor.matmul(
                        rp[rows, :], ones_bf[0:1, 0:64], rec_tiles[slot][:],
                        start=True, stop=True,
                    )
                    nc.vector.tensor_mul(
                        attn_p[hp][qg][rows, :],
                        tmp[rows, :],
                        rp[rows, :],
                    )

            def outproj_chunk(qg):
                """partial out-projection rows for query group qg."""
                for tt4 in range(4):
                    tt = 4 * qg + tt4
                    o_sb = osb.tile([128, C], F32, tag="o", name="osb", bufs=2)
                    for cn in range(2):
                        op = ps_misc.tile(
                            [128, 512], F32, tag="sm", name="op"
                        )
                        for hp in range(2):
                            nc.tensor.matmul(
                                op[:],
                                attn_p[hp][qg][:, ts(tt4, 128)],
                                wout_sb[:, hp, ts(cn, 512)],
                                start=(hp == 0),
                                stop=(hp == 1),
                            )
                        nc.vector.tensor_copy(o_sb[:, ts(cn, 512)], op[:])
                    nc.sync.dma_start(out=out[ts(tt, 128), :], in_=o_sb[:])

            # ---- flash-style startup: feed ACT as early as possible ----
            # Scores-critical work is emitted (= prioritized) strictly ahead
            # of the V projections, which are pure PE filler; v runs in the
            # ACT-paced slack of rounds 1-2 and must finish before pv(0,0)
            # releases round-0's p-tiles for round 2's exps.
            rounds = [(qg, hp) for qg in range(4) for hp in range(2)]

            # round 0 (qg0, hp0): k chunk0 + q chunk0(tt0), scores chase the
            # k tt-groups as they land.
            qk_group(1, 0, 0)
            qk_group(0, 0, 0)
            pa0, pb0 = p_alloc(0, 0)
            st_seg(0, 0, [0, 1], pa0, pb0)
            qk_group(1, 0, 1)
            st_seg(0, 0, [2, 3], pa0, pb0)
            qk_group(1, 0, 2)
            st_seg(0, 0, [4, 5], pa0, pb0)
            qk_group(1, 0, 3)
            st_seg(0, 0, [6, 7], pa0, pb0)

            # round 1 (qg0, hp1): k chunk1 + q chunk1(tt0)
            qk_group(1, 1, 0)
            qk_group(0, 1, 0)
            pa1, pb1 = p_alloc(0, 1)
            st_seg(0, 1, [0, 1], pa1, pb1)
            qk_group(1, 1, 1)
            st_seg(0, 1, [2, 3], pa1, pb1)
            qk_group(1, 1, 2)
            st_seg(0, 1, [4, 5], pa1, pb1)
            qk_group(1, 1, 3)
            st_seg(0, 1, [6, 7], pa1, pb1)

            # round 2 (qg1, hp0) scores, then V in its ACT-slack
            qk_group(0, 0, 1)
            qk_group(0, 1, 1)
            st_part(1, 0)
            for tt in range(16):
                v_group(tt)

            # ---- pipelined main stream ----
            pv_part(0, 0)
            st_part(1, 1)
            pv_part(0, 1)
            qk_group(0, 0, 2)
            qk_group(0, 1, 2)
            st_part(2, 0)
            normalize_round(0, 0)
            pv_part(1, 0)
            st_part(2, 1)
            normalize_round(0, 1)
            outproj_chunk(0)
            pv_part(1, 1)
            qk_group(0, 0, 3)
            qk_group(0, 1, 3)
            st_part(3, 0)
            normalize_round(1, 0)
            pv_part(2, 0)
            st_part(3, 1)
            normalize_round(1, 1)
            outproj_chunk(1)
            pv_part(2, 1)
            normalize_round(2, 0)
            pv_part(3, 0)
            normalize_round(2, 1)
            outproj_chunk(2)
            pv_part(3, 1)
            normalize_round(3, 0)
            normalize_round(3, 1)
            outproj_chunk(3)

    nc.compile()
    return nc


def _get_nc():
    global _NC_CACHE
    if _NC_CACHE is None:
        _NC_CACHE = _build_nc()
    return _NC_CACHE


def kernel(x, w_qkv, b_qkv, w_out, b_out):
    global LAST_RESULT
    x = np.asarray(x, dtype=np.float32)
    w_qkv = np.asarray(w_qkv, dtype=np.float32)
    b_qkv = np.asarray(b_qkv, dtype=np.float32)
    w_out = np.asarray(w_out, dtype=np.float32)
    b_out = np.asarray(b_out, dtype=np.float32)

    bf = ml_dtypes.bfloat16
    in_maps = []
    for c in range(N_CORES):
        b, g = divmod(c, 4)
        cols = slice(CH * g, CH * (g + 1))
        bq = b_qkv[0 * C + CH * g : 0 * C + CH * (g + 1)]
        bk = b_qkv[1 * C + CH * g : 1 * C + CH * (g + 1)]
        in_maps.append(
            {
                "xt": np.ascontiguousarray(x[b].astype(bf).T),
                "wq": np.ascontiguousarray(w_qkv[:, 0 * C :][:, cols]).astype(bf),
                "wk": np.ascontiguousarray(w_qkv[:, 1 * C :][:, cols]).astype(bf),
                "wv": np.ascontiguousarray(w_qkv[:, 2 * C :][:, cols]).astype(bf),
                "bqt": np.ascontiguousarray(bq.reshape(2, 128).T),
                "bkt": np.ascontiguousarray(bk.reshape(2, 128).T),
                "wout": np.ascontiguousarray(w_out[CH * g : CH * (g + 1), :]).astype(bf),
            }
        )

    nc = _get_nc()
    LAST_RESULT = bass_utils.run_bass_kernel_spmd(
        nc, in_maps, core_ids=list(range(N_CORES))
    )

    full = np.zeros((B, T, C), dtype=np.float32)
    # bias folded once on the host: b_out plus the V-bias pushed through
    # w_out (normalized attention rows sum to 1, so bv contributes exactly
    # bv @ w_out to every token)
    full += b_out + b_qkv[2 * C : 3 * C] @ w_out
    for c in range(N_CORES):
        b = c // 4
        full[b] += LAST_RESULT.results[c]["out"]
    return full


# revision 13
# speedup vs baseline: 1.0168x; 1.0067x over previous
"""Multi-head self-attention on 8 Trainium2 NeuronCores (Bass/Tile).

Problem: x[2,2048,1024] -> MHA(16 heads, d_head 64) -> out[2,2048,1024].

Sharding (batch x head-group, Megatron-ish, collective-free):
  core c (0..7): batch b = c//4, head group g = c%4 (heads 4g..4g+3).
  Each core computes q/k/v projections for its 4 heads over its batch,
  attention for those heads, and a PARTIAL output projection
  attn_local[256ch] @ w_out[256ch rows] over the full sequence. The host
  sums the 4 partials per batch (the Megatron row-parallel all-reduce is
  folded into the unshard step; b_out and the V-bias term bv @ w_out are
  added once on the host -- exact, since softmax rows sum to 1).

On-core layout (TensorE compute in bf16, fp32 PSUM accumulation):
  - ACT (exp for softmax) is the bottleneck engine: 16.8M exps/core ~=
    147us of ACT instruction time. The schedule saturates ACT from the
    earliest possible point after the ~7us engine-boot preamble:
    * all inputs arrive in host-pre-blocked layouts that are contiguous
      per SBUF partition (cheap HWDGE triggers, 4-8KB DMA lines), split
      across the sync and scalar queues with x^T token-sliced;
    * k/q chunk-0 projections are emitted first so scores round 0 feeds
      ACT immediately; V projections and the remaining q/k chunks are
      pure PE filler behind ACT pacing.
  - exp p-tiles are paired per round ([128, 2heads, 8kp, 1024]) with 3
    rotating buffers = 3 rounds in flight, so round r+2's exps never
    wait on round r's PV consumption.
  - qT/kT in [channel, t] layout: scores^T = kT.T @ qT with the two
    heads of a chunk in partitions 0-63/64-127 -> concurrent K=64
    matmuls in disjoint PE row groups.
  - softmax: scores^T [128ki, qi] -> ACT exp (PSUM->SBUF bf16,
    scale=1/8 folded, no max subtraction: |s|/8 <= ~2).
  - PV: attn^T = V.T @ P~ as column-tiled concurrent M=64 head pairs;
    denominators via DVE bf16 add-tree + K=128 ones-matmul fold,
    reciprocal_approx_fast straight off PSUM; normalize one round
    behind PV (rep-matmul broadcast + DVE mul); out-projection +
    output DMA per query group.
"""

import numpy as np
import ml_dtypes

import concourse.bass as bass
import concourse.mybir as mybir
import concourse.tile as tile
from concourse import bacc
from concourse import bass_utils
from concourse.bass import ts

BF = mybir.dt.bfloat16
F32 = mybir.dt.float32

B, T, C = 2, 2048, 1024
H, DH = 16, 64
N_CORES = 8
HG = 4  # heads per core
CH = HG * DH  # 256 channels per core

LAST_RESULT = None  # BassKernelResults of the most recent run (for profiling)
_NC_CACHE = None


def _build_nc():
    nc = bacc.Bacc(
        "TRN2", target_bir_lowering=False, debug=False, num_devices=N_CORES
    )

    # host-pre-blocked layouts: every tensor is contiguous along its SBUF
    # partition's free dim, so each DMA is 128 x (one fat line).
    xt = nc.dram_tensor("xt", [128, 4, 8, 512], BF, kind="ExternalInput")
    wq = nc.dram_tensor("wq", [128, 8, CH], BF, kind="ExternalInput")
    wk = nc.dram_tensor("wk", [128, 8, CH], BF, kind="ExternalInput")
    wv = nc.dram_tensor("wv", [128, 8, CH], BF, kind="ExternalInput")
    bqt = nc.dram_tensor("bqt", [128, 2], F32, kind="ExternalInput")
    bkt = nc.dram_tensor("bkt", [128, 2], F32, kind="ExternalInput")
    wout = nc.dram_tensor("wout", [128, 2, C], BF, kind="ExternalInput")
    out = nc.dram_tensor("out", [T, C], F32, kind="ExternalOutput")

    with tile.TileContext(nc) as tc:
        with (
            tc.tile_pool(name="persist", bufs=1) as persist,
            tc.tile_pool(name="consts", bufs=1) as consts,
            tc.tile_pool(name="sbn", bufs=2) as sbn,
            tc.tile_pool(name="osb", bufs=3) as osb,
            tc.tile_pool(name="ps_st", bufs=2, space="PSUM") as ps_st,
            tc.tile_pool(name="ps_pv", bufs=2, space="PSUM") as ps_pv,
            tc.tile_pool(name="ps_misc", bufs=2, space="PSUM") as ps_misc,
        ):
            ones_bf = consts.tile([1, 128], BF)
            nc.vector.memset(ones_bf[:], 1.0)
            ones_col = consts.tile([128, 1], BF)
            nc.vector.memset(ones_col[:], 1.0)

            # xT is token-major: [p, tt(512-token block), ci, t-within-block]
            xT = persist.tile([128, 4, 8, 512], BF, tag="xT")
            wq_sb = persist.tile([128, 8, CH], BF, tag="wq")
            wk_sb = persist.tile([128, 8, CH], BF, tag="wk")
            wv_sb = persist.tile([128, 8, CH], BF, tag="wv")
            wout_sb = persist.tile([128, 2, C], BF, tag="wout")
            bqt_sb = consts.tile([128, 2], F32)
            bkt_sb = consts.tile([128, 2], F32)

            # ---- input DMA: sync + scalar HWDGE queues, critical-first ----
            nc.sync.dma_start(out=wk_sb[:], in_=wk[:])
            nc.scalar.dma_start(out=bkt_sb[:], in_=bkt[:])
            nc.scalar.dma_start(out=bqt_sb[:], in_=bqt[:])
            nc.scalar.dma_start(out=wq_sb[:], in_=wq[:])
            nc.sync.dma_start(out=xT[:, 0], in_=xt[:, 0])
            nc.scalar.dma_start(out=xT[:, 1], in_=xt[:, 1])
            nc.sync.dma_start(out=xT[:, 2], in_=xt[:, 2])
            nc.scalar.dma_start(out=xT[:, 3], in_=xt[:, 3])
            nc.gpsimd.dma_start(out=wv_sb[:], in_=wv[:])
            nc.gpsimd.dma_start(out=wout_sb[:], in_=wout[:])

            # ---- persistent activations ----
            # qkT[:, 0:2, :] = qT chunks (hp), [:, 2:4, :] = kT chunks;
            # chunk hp rows 0-63 = head 2hp, rows 64-127 = head 2hp+1.
            qkT = persist.tile([128, 4, T], BF, tag="qkT")
            vext = persist.tile([128, T // 128, HG, DH], BF, tag="vext")
            attn_p = [
                [
                    persist.tile(
                        [128, 512], BF, tag=f"attnp{hp}_{qg}",
                        name=f"attnp{hp}_{qg}",
                    )
                    for qg in range(4)
                ]
                for hp in range(2)
            ]

            def qk_group(w_i, co, tt):
                """one [128,512] tile of qT (w_i=0) or kT (w_i=1), chunk co"""
                wsb = wq_sb if w_i == 0 else wk_sb
                bias_sb = bqt_sb if w_i == 0 else bkt_sb
                qp = ps_misc.tile([128, 512], F32, tag="sm", name="qp")
                for ci in range(8):
                    nc.tensor.matmul(
                        qp[:],
                        wsb[:, ci, ts(co, 128)],
                        xT[:, tt, ci, :],
                        start=(ci == 0),
                        stop=(ci == 7),
                    )
                # bias-add + cast on the DVE (keeps the ACT queue for exps)
                nc.vector.tensor_scalar_add(
                    qkT[:, 2 * w_i + co, ts(tt, 512)],
                    qp[:],
                    bias_sb[:, co : co + 1],
                )

            def v_group(tv):
                vp = ps_misc.tile([128, CH], F32, tag="sm", name="vp")
                for ci in range(8):
                    nc.tensor.matmul(
                        vp[:],
                        xT[:, tv // 4, ci, ts(tv % 4, 128)],
                        wv_sb[:, ci, :],
                        start=(ci == 0),
                        stop=(ci == 7),
                    )
                nc.vector.tensor_copy(
                    vext[:, tv, :, :],
                    vp[:].rearrange("p (h d) -> p h d", h=HG),
                )

            p_tiles = {}
            rec_tiles = {}
            tmp_tiles = {}

            def p_alloc(qg, hp):
                # paired tile: [p, head(A/B), kp, 1024]; one pool slot per
                # round -> bufs=3 keeps 3 rounds of exps live.
                p = osb.tile([128, 2, 8, 1024], BF, tag="p", bufs=3, name="p")
                p_tiles[(qg, hp)] = p
                return p

            def st_seg(qg, hp, kps, p):
                """scores^T + exp for head pair hp, query group qg, kp range."""
                qs = ts(qg, 512)
                for kp in kps:
                    stA = ps_st.tile([128, 1024], F32, tag="st", name="stA")
                    stB = ps_st.tile([128, 1024], F32, tag="st", name="stB")
                    for j in range(2):
                        ki = 2 * kp + j
                        nc.tensor.matmul(
                            stA[:, ts(j, 512)],
                            qkT[0:64, 2 + hp, ts(ki, 128)],
                            qkT[0:64, hp, qs],
                            start=True, stop=True,
                        )
                        nc.tensor.matmul(
                            stB[:, ts(j, 512)],
                            qkT[64:128, 2 + hp, ts(ki, 128)],
                            qkT[64:128, hp, qs],
                            start=True, stop=True,
                        )
                    nc.scalar.activation(
                        p[:, 0, kp, :], stA[:],
                        mybir.ActivationFunctionType.Exp, scale=1.0 / 8.0,
                    )
                    nc.scalar.activation(
                        p[:, 1, kp, :], stB[:],
                        mybir.ActivationFunctionType.Exp, scale=1.0 / 8.0,
                    )

            def st_part(qg, hp):
                p = p_alloc(qg, hp)
                st_seg(qg, hp, range(8), p)

            def pv_part(qg, hp):
                p = p_tiles.pop((qg, hp))
                # denominator add-trees first: they depend only on the exps,
                # so emitting them ahead of the PV matmuls keeps the DVE
                # queue from head-blocking on the PV-dependent tmp copy.
                t4s = {}
                for hh in range(2):
                    t1 = sbn.tile([128, 4, 1024], BF, tag="t1", name="t1", bufs=1)
                    nc.vector.tensor_add(
                        t1[:], p[:, hh, 0:4, :], p[:, hh, 4:8, :]
                    )
                    t2 = sbn.tile([128, 2, 1024], BF, tag="t2", name="t2", bufs=1)
                    nc.vector.tensor_add(
                        t2[:], t1[:, 0:2, :], t1[:, 2:4, :]
                    )
                    t3 = sbn.tile([128, 1024], BF, tag="t3", name="t3", bufs=1)
                    nc.vector.tensor_add(
                        t3[:], t2[:, 0, :], t2[:, 1, :]
                    )
                    t4 = sbn.tile([128, 512], BF, tag="t4", name="t4", bufs=2)
                    nc.vector.tensor_add(
                        t4[:], t3[:, 0:512], t3[:, 512:1024]
                    )
                    t4s[hh] = t4
                # paired PV: head 2hp -> psum partitions 0-63 (col group 0-1),
                # head 2hp+1 -> partitions 64-127 (col group 2-3); the two
                # column-tiled matmul streams run concurrently on the PE.
                pv = ps_pv.tile([128, 512], F32, tag="pv", name="pv")
                for ki in range(16):
                    for hh in range(2):
                        h = 2 * hp + hh
                        nc.tensor.matmul(
                            pv[64 * hh : 64 * hh + 64, :],
                            vext[:, ki, h, :],
                            p[:, hh, ki // 2, ts(ki % 2, 512)],
                            start=(ki == 0),
                            stop=(ki == 15),
                        )
                # partition-axis fold of the partial denominators (K=128
                # ones-matmul), then the reciprocal chain (DVE-only)
                for hh in range(2):
                    h = 2 * hp + hh
                    dps = ps_misc.tile([128, 512], F32, tag="sm", name="dps")
                    nc.tensor.matmul(
                        dps[0:1, :], ones_col[:, 0:1], t4s[hh][:],
                        start=True, stop=True,
                    )
                    rec32 = sbn.tile([1, 512], F32, tag="rec32", name="rc", bufs=1)
                    nc.vector.reciprocal_approx_fast(out=rec32[:], in_=dps[0:1, :])
                    rec_bf = sbn.tile([1, 512], BF, tag="rec", name="rb", bufs=4)
                    nc.vector.tensor_copy(rec_bf[:], rec32[:])
                    rec_tiles[4 * qg + h] = rec_bf
                tmp = sbn.tile([128, 512], BF, tag="tmp", name="tmp", bufs=4)
                nc.vector.tensor_copy(tmp[:], pv[:])
                tmp_tiles[(qg, hp)] = tmp

            def normalize_round(qg, hp):
                """rep-matmul + multiply -> attn_p[hp][qg] (both heads)."""
                rp = ps_misc.tile([128, 512], F32, tag="sm", name="rp")
                tmp = tmp_tiles.pop((qg, hp))
                for hh in range(2):
                    slot = 4 * qg + 2 * hp + hh
                    rows = slice(64 * hh, 64 * hh + 64)
                    nc.tensor.matmul(
                        rp[rows, :], ones_bf[0:1, 0:64], rec_tiles[slot][:],
                        start=True, stop=True,
                    )
                    nc.vector.tensor_mul(
                        attn_p[hp][qg][rows, :],
                        tmp[rows, :],
                        rp[rows, :],
                    )

            def outproj_chunk(qg):
                """partial out-projection rows for query group qg."""
                for tt4 in range(4):
                    tt = 4 * qg + tt4
                    o_sb = osb.tile([128, C], F32, tag="o", name="osb", bufs=1)
                    for cn in range(2):
                        op = ps_misc.tile(
                            [128, 512], F32, tag="sm", name="op"
                        )
                        for hp in range(2):
                            nc.tensor.matmul(
                                op[:],
                                attn_p[hp][qg][:, ts(tt4, 128)],
                                wout_sb[:, hp, ts(cn, 512)],
                                start=(hp == 0),
                                stop=(hp == 1),
                            )
                        nc.vector.tensor_copy(o_sb[:, ts(cn, 512)], op[:])
                    nc.sync.dma_start(out=out[ts(tt, 128), :], in_=o_sb[:])

            # ---- flash-style startup: feed ACT as early as possible ----
            # Scores-critical work is emitted (= prioritized) strictly ahead
            # of the V projections, which are pure PE filler in the ACT-paced
            # slack of rounds 1-2; pv(0,0) directly follows V.
            # round 0 (qg0, hp0): k chunk0 + q chunk0(tt0); scores chase the
            # k tt-groups as they land.
            qk_group(1, 0, 0)
            qk_group(0, 0, 0)
            p00 = p_alloc(0, 0)
            st_seg(0, 0, [0, 1], p00)
            qk_group(1, 0, 1)
            st_seg(0, 0, [2, 3], p00)
            qk_group(1, 0, 2)
            st_seg(0, 0, [4, 5], p00)
            qk_group(1, 0, 3)
            st_seg(0, 0, [6, 7], p00)

            # round 1 (qg0, hp1): k chunk1 + q chunk1(tt0)
            qk_group(1, 1, 0)
            qk_group(0, 1, 0)
            p01 = p_alloc(0, 1)
            st_seg(0, 1, [0, 1], p01)
            qk_group(1, 1, 1)
            st_seg(0, 1, [2, 3], p01)
            qk_group(1, 1, 2)
            st_seg(0, 1, [4, 5], p01)
            qk_group(1, 1, 3)
            st_seg(0, 1, [6, 7], p01)

            # round 2 (qg1, hp0) scores, then V in its ACT-slack
            qk_group(0, 0, 1)
            qk_group(0, 1, 1)
            st_part(1, 0)
            for tv in range(16):
                v_group(tv)

            # ---- pipelined main stream ----
            pv_part(0, 0)
            st_part(1, 1)
            pv_part(0, 1)
            qk_group(0, 0, 2)
            qk_group(0, 1, 2)
            st_part(2, 0)
            normalize_round(0, 0)
            pv_part(1, 0)
            st_part(2, 1)
            normalize_round(0, 1)
            outproj_chunk(0)
            pv_part(1, 1)
            qk_group(0, 0, 3)
            qk_group(0, 1, 3)
            st_part(3, 0)
            normalize_round(1, 0)
            pv_part(2, 0)
            st_part(3, 1)
            normalize_round(1, 1)
            outproj_chunk(1)
            pv_part(2, 1)
            normalize_round(2, 0)
            pv_part(3, 0)
            normalize_round(2, 1)
            outproj_chunk(2)
            pv_part(3, 1)
            normalize_round(3, 0)
            normalize_round(3, 1)
            outproj_chunk(3)

    nc.compile()
    return nc


def _get_nc():
    global _NC_CACHE
    if _NC_CACHE is None:
        _NC_CACHE = _build_nc()
    return _NC_CACHE


def kernel(x, w_qkv, b_qkv, w_out, b_out):
    global LAST_RESULT
    x = np.asarray(x, dtype=np.float32)
    w_qkv = np.asarray(w_qkv, dtype=np.float32)
    b_qkv = np.asarray(b_qkv, dtype=np.float32)
    w_out = np.asarray(w_out, dtype=np.float32)
    b_out = np.asarray(b_out, dtype=np.float32)

    bf = ml_dtypes.bfloat16

    def blk_w(w):  # [1024, n] -> [128, 8, n] (p, ci, n) contiguous
        n = w.shape[1]
        return np.ascontiguousarray(
            w.reshape(8, 128, n).transpose(1, 0, 2)
        ).astype(bf)

    in_maps = []
    for c in range(N_CORES):
        b, g = divmod(c, 4)
        cols = slice(CH * g, CH * (g + 1))
        bq = b_qkv[0 * C + CH * g : 0 * C + CH * (g + 1)]
        bk = b_qkv[1 * C + CH * g : 1 * C + CH * (g + 1)]
        # x^T token-blocked: [p, tt, ci, 512]
        xtb = np.ascontiguousarray(
            x[b].T.astype(bf).reshape(8, 128, 4, 512).transpose(1, 2, 0, 3)
        )
        # wout row-blocked: [p, hp, 1024]
        wob = np.ascontiguousarray(
            w_out[CH * g : CH * (g + 1), :].reshape(2, 128, C).transpose(1, 0, 2)
        ).astype(bf)
        in_maps.append(
            {
                "xt": xtb,
                "wq": blk_w(w_qkv[:, 0 * C :][:, cols]),
                "wk": blk_w(w_qkv[:, 1 * C :][:, cols]),
                "wv": blk_w(w_qkv[:, 2 * C :][:, cols]),
                "bqt": np.ascontiguousarray(bq.reshape(2, 128).T),
                "bkt": np.ascontiguousarray(bk.reshape(2, 128).T),
                "wout": wob,
            }
        )

    nc = _get_nc()
    LAST_RESULT = bass_utils.run_bass_kernel_spmd(
        nc, in_maps, core_ids=list(range(N_CORES))
    )

    full = np.zeros((B, T, C), dtype=np.float32)
    # bias folded once on the host: b_out plus the V-bias pushed through
    # w_out (normalized attention rows sum to 1, so bv contributes exactly
    # bv @ w_out to every token)
    full += b_out + b_qkv[2 * C : 3 * C] @ w_out
    for c in range(N_CORES):
        b = c // 4
        full[b] += LAST_RESULT.results[c]["out"]
    return full
